# revision 10
# baseline (speedup 1.0000x reference)
"""Cross-scale attention kernel for Trainium2 (8 NeuronCores, SPMD).

Sharding: core c handles batch b = c//2 and query-half h = c%2. Each core
computes K_j/V_j for all keys of its batch (duplicated across the 2 cores of
a batch — cheap), and runs all 9 (i,j) attention pairs restricted to its half
of the query rows. Attention output for a query is invariant to key order, so
per-core inputs are column-permuted to put the core's query half first,
letting a single NEFF serve all 8 cores with no collectives.

Layouts (per core):
  X_i   = feat_i[b] as [C=256, N_i]  (channel-major, the native feat layout)
  Q^T_i = qw_i @ X_i[:, :NQ_i]       [256, NQ_i]
  K^T_j = kw_j @ X_j                 [256, N_j]
  V_j   = X_j^T @ vw_j^T (+ ones col)[N_j, 257]
  scores^T = (K^T)^T-tile @ Q^T      [keys, q]   (PE, f32r)
  E = exp(scores * SCALE)            (ACT, no max-subtraction needed: |s|<~2)
  AV: E_slice @ V_hat -> [q, 257]    (col 256 = softmax denominator, free)
  out-proj + residual + LayerNorm in [rows, 256] space, PE-transpose to
  channel-major for the output.
"""

import sys

for _p in ("/opt/trn_rl_repo",):
    if _p not in sys.path:
        sys.path.insert(0, _p)

import numpy as np
from contextlib import ExitStack

import concourse.bass as bass
import concourse.tile as tile
from concourse import bacc
from concourse import mybir
from concourse.masks import make_identity

P = 128
D = 256
S = 3
HW = [64, 32, 16]
NF = [4096, 1024, 256]          # full sequence lengths per scale
NQ = [2048, 512, 128]           # per-core query rows (half of NF)
B = 4
SCALE = float((D // 8) ** -0.5)
EPS = 1e-5
CK = 512                        # q-chunk width
F32 = mybir.dt.float32
F32R = mybir.dt.float32r
AF = mybir.ActivationFunctionType
ALU = mybir.AluOpType


def _r(ap):
    """View an fp32 AP as float32r for full-rate PE matmuls."""
    return ap.bitcast(F32R)


def _bcast(ap1d, p=P):
    """Partition-broadcast AP (stride-0 partition dim) for DMA replication."""
    return bass.AP(
        tensor=ap1d.tensor,
        offset=ap1d.offset,
        ap=[[0, p]] + [list(x) for x in ap1d.ap],
    )


def build_program():
    nc = bacc.Bacc("TRN2", target_bir_lowering=False, debug=False)

    x = [
        nc.dram_tensor(f"x{i}", [D, NF[i]], F32, kind="ExternalInput").ap()
        for i in range(S)
    ]
    w_dram = {}
    for nm in ("qw", "kw", "vw", "ow"):
        w_dram[nm] = nc.dram_tensor(nm, [S, D, D], F32, kind="ExternalInput").ap()
    b_dram = {}
    for nm in ("qb", "kb", "vb", "ob", "ln_g", "ln_b"):
        b_dram[nm] = nc.dram_tensor(nm, [S, D], F32, kind="ExternalInput").ap()
    y = [
        nc.dram_tensor(f"y{i}", [D, NQ[i]], F32, kind="ExternalOutput").ap()
        for i in range(S)
    ]

    with tile.TileContext(nc) as tc, ExitStack() as ctx:
        _emit(tc, ctx, x, w_dram, b_dram, y)
    nc.compile()
    return nc


def _emit(tc, ctx, x, w_dram, b_dram, y):
    nc = tc.nc

    singles = ctx.enter_context(tc.tile_pool(name="singles", bufs=1))
    tpsum = ctx.enter_context(tc.tile_pool(name="tpsum", bufs=2, space="PSUM"))
    spsum = ctx.enter_context(tc.tile_pool(name="spsum", bufs=2, space="PSUM"))
    apsum = ctx.enter_context(tc.tile_pool(name="apsum", bufs=4, space="PSUM"))

    ident = singles.tile([P, P], F32, tag="ident")
    make_identity(nc, ident)

    eps_t = singles.tile([P, 1], F32, tag="eps")
    nc.vector.memset(eps_t, EPS)

    # ---- biases ----
    # per-partition layout (value indexed by output-channel on partitions)
    bk = {}
    for nm in ("qb", "kb", "ob", "ln_g", "ln_b"):
        for s in range(S):
            t = singles.tile([P, 2], F32, tag=f"bk_{nm}{s}")
            nc.gpsimd.dma_start(
                out=t, in_=b_dram[nm][s].rearrange("(po pi) -> pi po", pi=P)
            )
            bk[(nm, s)] = t
    # partition-broadcast layout (value indexed along the free dim)
    bb = {}
    for nm in ("vb", "ob", "ln_g", "ln_b"):
        for s in range(S):
            t = singles.tile([P, D], F32, tag=f"bb_{nm}{s}")
            nc.gpsimd.dma_start(out=t, in_=_bcast(b_dram[nm][s]))
            bb[(nm, s)] = t

    # ---- transposed weights:  wT[(kind,s)] is [Din_pi, Din_po, Dout] ----
    with tc.tile_pool(name="wstage", bufs=2) as wstage, \
            tc.tile_pool(name="xstage", bufs=1) as xstage:
        wT = {}
        for kind in ("qw", "kw", "vw", "ow"):
            for s in range(S):
                wsb = wstage.tile([P, 2, D], F32, tag="wload")
                nc.sync.dma_start(
                    out=wsb,
                    in_=w_dram[kind][s].rearrange("(po pi) f -> pi po f", pi=P),
                )
                wt = singles.tile([P, 2, D], F32R, tag=f"wT_{kind}{s}")
                for a in range(2):       # Din outer
                    for bo in range(2):  # Dout outer
                        pt = tpsum.tile([P, 512], F32, tag="t", name="t")[:, :P]
                        nc.tensor.transpose(pt, wsb[:, bo, a * P:(a + 1) * P], ident)
                        if kind == "ow":
                            # fold the "combined / S" into the output proj
                            nc.vector.tensor_scalar_mul(
                                out=wt[:, a, bo * P:(bo + 1) * P], in0=pt,
                                scalar1=1.0 / S,
                            )
                        else:
                            nc.vector.tensor_copy(
                                out=wt[:, a, bo * P:(bo + 1) * P], in_=pt
                            )
                wT[(kind, s)] = wt

        # ---- projections ----
        kT, vh, qT = {}, {}, {}
        for s in range(S):
            # DMA X in chunks and round each into a persistent f32r tile:
            # the BIR verifier requires every writer of an f32r-matmul operand
            # location to be a compute op with f32r output (DMA can't round).
            xs = xstage.tile([P, 2, NF[s]], F32R, tag="x")
            for c0 in range(0, NF[s], CK):
                w = min(CK, NF[s] - c0)
                xld = wstage.tile([P, 2, CK], F32, tag="xld", name="xld")[:, :, :w]
                nc.sync.dma_start(
                    out=xld,
                    in_=x[s].rearrange("(po pi) n -> pi po n", pi=P)[:, :, c0:c0 + w],
                )
                nc.vector.tensor_copy(out=xs[:, :, c0:c0 + w], in_=xld)

            # K^T_s : [Dout_pi, Dout_po, N]
            kt = singles.tile([P, 2, NF[s]], F32R, tag=f"kT{s}")
            for dt_ in range(2):
                for c0 in range(0, NF[s], CK):
                    w = min(CK, NF[s] - c0)
                    ps = tpsum.tile([P, 512], F32, tag="t", name="t")[:, :w]
                    for a in range(2):
                        nc.tensor.matmul(
                            ps,
                            lhsT=wT[("kw", s)][:, a, dt_ * P:(dt_ + 1) * P],
                            rhs=_r(xs[:, a, c0:c0 + w]),
                            start=(a == 0),
                            stop=(a == 1),
                        )
                    nc.vector.tensor_scalar_add(
                        out=kt[:, dt_, c0:c0 + w],
                        in0=ps,
                        scalar1=bk[("kb", s)][:, dt_:dt_ + 1],
                    )
            kT[s] = kt

            # V_s padded to 258 cols: col 256 = ones (softmax denominator
            # accumulator), col 257 = zeros (fp32r matmuls need an even
            # innermost dst count, and f32r Memset is not a valid ISA inst,
            # so both extras are written via tensor_scalar).
            vt = singles.tile([P, NF[s] // P, D + 2], F32R, tag=f"v{s}")
            for rt in range(NF[s] // P):
                ps = tpsum.tile([P, 512], F32, tag="t", name="t")[:, :D]
                for a in range(2):
                    nc.tensor.matmul(
                        ps,
                        lhsT=_r(xs[:, a, rt * P:(rt + 1) * P]),
                        rhs=wT[("vw", s)][:, a, :],
                        start=(a == 0),
                        stop=(a == 1),
                    )
                nc.vector.tensor_add(
                    out=vt[:, rt, :D], in0=ps, in1=bb[("vb", s)]
                )
                nc.vector.tensor_scalar(
                    out=vt[:, rt, D:D + 1], in0=eps_t,
                    scalar1=0.0, scalar2=1.0, op0=ALU.mult, op1=ALU.add,
                )
                nc.vector.tensor_scalar(
                    out=vt[:, rt, D + 1:D + 2], in0=eps_t,
                    scalar1=0.0, scalar2=None, op0=ALU.mult,
                )
            vh[s] = vt

            # Q^T_s : [Dout_pi, Dout_po, NQ]
            qt = singles.tile([P, 2, NQ[s]], F32R, tag=f"qT{s}")
            for dt_ in range(2):
                for c0 in range(0, NQ[s], CK):
                    w = min(CK, NQ[s] - c0)
                    ps = tpsum.tile([P, 512], F32, tag="t", name="t")[:, :w]
                    for a in range(2):
                        nc.tensor.matmul(
                            ps,
                            lhsT=wT[("qw", s)][:, a, dt_ * P:(dt_ + 1) * P],
                            rhs=_r(xs[:, a, c0:c0 + w]),
                            start=(a == 0),
                            stop=(a == 1),
                        )
                    nc.vector.tensor_scalar_add(
                        out=qt[:, dt_, c0:c0 + w],
                        in0=ps,
                        scalar1=bk[("qb", s)][:, dt_:dt_ + 1],
                    )
            qT[s] = qt

    # ---- attention + output ----
    etp = ctx.enter_context(tc.tile_pool(name="etp", bufs=3))
    combp = ctx.enter_context(tc.tile_pool(name="combp", bufs=2))
    outp = ctx.enter_context(tc.tile_pool(name="outp", bufs=2))
    miscp = ctx.enter_context(tc.tile_pool(name="miscp", bufs=4))

    for i in range(S):
        W = min(CK, NQ[i])
        for c0 in range(0, NQ[i], W):
            nqs = W // P
            comb = combp.tile([P, 4, D], F32, tag="comb", name="comb")[:, :nqs, :]

            for j in range(S):
                ktiles = NF[j] // P
                avp = [
                    apsum.tile([P, 512], F32, tag="av", name="av")[:, :D + 2]
                    for _ in range(nqs)
                ]
                for kt_ in range(ktiles):
                    sps = spsum.tile([P, 512], F32, tag="s", name="s")[:, :W]
                    for a in range(2):
                        nc.tensor.matmul(
                            sps,
                            lhsT=kT[j][:, a, kt_ * P:(kt_ + 1) * P],
                            rhs=qT[i][:, a, c0:c0 + W],
                            start=(a == 0),
                            stop=(a == 1),
                        )
                    et = etp.tile([P, 512], F32R, tag="et", name="et")[:, :W]
                    nc.scalar.activation(out=et, in_=sps, func=AF.Exp, scale=SCALE)
                    for qs in range(nqs):
                        nc.tensor.matmul(
                            avp[qs],
                            lhsT=et[:, qs * P:(qs + 1) * P],
                            rhs=vh[j][:, kt_, :],
                            start=(kt_ == 0),
                            stop=(kt_ == ktiles - 1),
                        )
                for qs in range(nqs):
                    rec = miscp.tile([P, 1], F32, tag="rec")
                    nc.vector.reciprocal(out=rec, in_=avp[qs][:, D:D + 1])
                    if j == 0:
                        nc.vector.tensor_scalar_mul(
                            out=comb[:, qs, :], in0=avp[qs][:, :D], scalar1=rec
                        )
                    else:
                        tmp = miscp.tile([P, D], F32, tag="ntmp")
                        nc.vector.tensor_scalar_mul(
                            out=tmp, in0=avp[qs][:, :D], scalar1=rec
                        )
                        nc.vector.tensor_add(
                            out=comb[:, qs, :], in0=comb[:, qs, :], in1=tmp
                        )

            # ---- output projection + residual + LN for this chunk ----
            cT = outp.tile([P, 2, CK], F32R, tag="cT", name="cT")[:, :, :W]
            for qs in range(nqs):
                for a in range(2):
                    pt = tpsum.tile([P, 512], F32, tag="t", name="t")[:, :P]
                    nc.tensor.transpose(pt, comb[:, qs, a * P:(a + 1) * P], ident)
                    nc.vector.tensor_copy(
                        out=cT[:, a, qs * P:(qs + 1) * P], in_=pt
                    )
            xr = outp.tile([P, 2, CK], F32, tag="xr", name="xr")[:, :, :W]
            nc.sync.dma_start(
                out=xr,
                in_=x[i].rearrange("(po pi) n -> pi po n", pi=P)[:, :, c0:c0 + W],
            )
            ysb = outp.tile([P, 2, CK], F32, tag="ysb", name="ysb")[:, :, :W]
            for qs in range(nqs):
                ops = tpsum.tile([P, 512], F32, tag="t", name="t")[:, :D]
                for a in range(2):
                    nc.tensor.matmul(
                        ops,
                        lhsT=cT[:, a, qs * P:(qs + 1) * P],
                        rhs=wT[("ow", i)][:, a, :],
                        start=(a == 0),
                        stop=(a == 1),
                    )
                o = miscp.tile([P, D], F32, tag="o")
                nc.vector.tensor_add(out=o, in0=ops, in1=bb[("ob", i)])
                for a in range(2):
                    xt = tpsum.tile([P, 512], F32, tag="t", name="t")[:, :P]
                    nc.tensor.transpose(xt, xr[:, a, qs * P:(qs + 1) * P], ident)
                    nc.vector.tensor_add(
                        out=o[:, a * P:(a + 1) * P],
                        in0=o[:, a * P:(a + 1) * P],
                        in1=xt,
                    )
                # LayerNorm over the free dim
                stats = miscp.tile([P, 6], F32, tag="st")
                nc.vector.bn_stats(out=stats, in_=o)
                mv = miscp.tile([P, 2], F32, tag="mv")
                nc.vector.bn_aggr(out=mv, in_=stats)
                nc.scalar.activation(
                    out=mv[:, 1:2], in_=mv[:, 1:2], func=AF.Sqrt,
                    bias=eps_t, scale=1.0,
                )
                nc.vector.reciprocal(out=mv[:, 1:2], in_=mv[:, 1:2])
                nc.vector.tensor_scalar(
                    out=o, in0=o,
                    scalar1=mv[:, 0:1], scalar2=mv[:, 1:2],
                    op0=ALU.subtract, op1=ALU.mult,
                )
                nc.vector.tensor_mul(out=o, in0=o, in1=bb[("ln_g", i)])
                nc.vector.tensor_add(out=o, in0=o, in1=bb[("ln_b", i)])
                for a in range(2):
                    yt = tpsum.tile([P, 512], F32, tag="t", name="t")[:, :P]
                    nc.tensor.transpose(yt, o[:, a * P:(a + 1) * P], ident)
                    nc.vector.tensor_copy(
                        out=ysb[:, a, qs * P:(qs + 1) * P], in_=yt
                    )
            nc.sync.dma_start(
                out=y[i].rearrange("(po pi) n -> pi po n", pi=P)[:, :, c0:c0 + W],
                in_=ysb,
            )


# ---------------------------------------------------------------------------
# host-side sharding / execution
# ---------------------------------------------------------------------------

_CACHE = {}


def _shard_inputs(inputs):
    feats = [np.ascontiguousarray(np.asarray(inputs[f"feat{i}"], np.float32))
             for i in range(S)]
    consts = {
        nm: np.ascontiguousarray(np.asarray(inputs[nm], np.float32))
        for nm in ("qw", "kw", "vw", "ow", "qb", "kb", "vb", "ob", "ln_g", "ln_b")
    }
    in_maps = []
    for c in range(8):
        b, h = c // 2, c % 2
        m = dict(consts)
        for i in range(S):
            xi = feats[i][b].reshape(D, NF[i])
            if h == 1:
                xi = np.concatenate([xi[:, NF[i] // 2:], xi[:, :NF[i] // 2]], axis=1)
            m[f"x{i}"] = np.ascontiguousarray(xi)
        in_maps.append(m)
    return in_maps


def _unshard_outputs(results):
    outs = []
    for i in range(S):
        full = np.empty((B, D, NF[i]), np.float32)
        for c in range(8):
            b, h = c // 2, c % 2
            full[b][:, h * NQ[i]:(h + 1) * NQ[i]] = results[c][f"y{i}"]
        outs.append(full.reshape(B, D, HW[i], HW[i]))
    return tuple(outs)


def get_nc():
    if "nc" not in _CACHE:
        _CACHE["nc"] = build_program()
    return _CACHE["nc"]


def _get_runner():
    """Build (once) a jitted 8-core SPMD executor for the program.

    Mirrors bass2jax.run_bass_via_pjrt's multi-core path, but caches the
    jitted callable so repeat kernel() calls don't re-trace/re-compile.
    No donation: the kernel writes every output element, so fresh result
    buffers are fine.
    """
    if "runner" in _CACHE:
        return _CACHE["runner"]

    import jax
    from jax.sharding import Mesh, PartitionSpec
    from jax.experimental.shard_map import shard_map
    from concourse import bass2jax, mybir

    nc = get_nc()
    bass2jax.install_neuronx_cc_hook()

    partition_name = (
        nc.partition_id_tensor.name if nc.partition_id_tensor else None
    )
    in_names, out_names, out_avals, zero_outs = [], [], [], []
    for alloc in nc.m.functions[0].allocations:
        if not isinstance(alloc, mybir.MemoryLocationSet):
            continue
        name = alloc.memorylocations[0].name
        if alloc.kind == "ExternalInput":
            if name != partition_name:
                in_names.append(name)
        elif alloc.kind == "ExternalOutput":
            shape = tuple(alloc.tensor_shape)
            dtype = mybir.dt.np(alloc.dtype)
            out_names.append(name)
            out_avals.append(jax.core.ShapedArray(shape, dtype))
            zero_outs.append(np.zeros(shape, dtype))
    n_params = len(in_names)
    all_names = in_names + out_names
    if partition_name is not None:
        all_names = all_names + [partition_name]

    def _body(*args):
        operands = list(args)
        if partition_name is not None:
            operands.append(bass2jax.partition_id_tensor())
        outs = bass2jax._bass_exec_p.bind(
            *operands,
            out_avals=tuple(out_avals),
            in_names=tuple(all_names),
            out_names=tuple(out_names),
            lowering_input_output_aliases=(),
            sim_require_finite=True,
            sim_require_nnan=True,
            nc=nc,
        )
        return tuple(outs)

    n_cores = 8
    devices = jax.devices()[:n_cores]
    mesh = Mesh(np.asarray(devices), ("core",))
    specs = (PartitionSpec("core"),) * (n_params + len(out_names))
    sharded = jax.jit(
        shard_map(
            _body, mesh=mesh,
            in_specs=specs,
            out_specs=(PartitionSpec("core"),) * len(out_names),
            check_rep=False,
        ),
        keep_unused=True,
    )

    concat_zeros = [
        np.zeros((n_cores * z.shape[0], *z.shape[1:]), z.dtype) for z in zero_outs
    ]
    zeros_dev = [jax.device_put(z, jax.NamedSharding(mesh, PartitionSpec("core")))
                 for z in concat_zeros]

    def prepare(in_maps):
        assert len(in_maps) == n_cores
        concat_in = [
            np.concatenate([np.asarray(in_maps[c][nm]) for c in range(n_cores)],
                           axis=0)
            for nm in in_names
        ]
        return [
            jax.device_put(a, jax.NamedSharding(mesh, PartitionSpec("core")))
            for a in concat_in
        ]

    def run(dev_in):
        out_arrs = sharded(*dev_in, *zeros_dev)
        jax.block_until_ready(out_arrs)
        return out_arrs

    def unpack(out_arrs):
        return [
            {
                nm: np.asarray(out_arrs[i]).reshape(n_cores, *out_avals[i].shape)[c]
                for i, nm in enumerate(out_names)
            }
            for c in range(n_cores)
        ]

    _CACHE["runner"] = (prepare, run, unpack)
    return _CACHE["runner"]


def kernel(**inputs):
    prepare, run, unpack = _get_runner()
    in_maps = _shard_inputs(inputs)
    results = unpack(run(prepare(in_maps)))
    return _unshard_outputs(results)


if __name__ == "__main__":
    nc = build_program()
    print("program built ok")


# revision 12
# speedup vs baseline: 3166.6286x; 3166.6286x over previous
"""Cross-scale attention kernel for Trainium2 (8 NeuronCores, SPMD).

Sharding: core c handles batch b = c//2 and query-half h = c%2. Each core
computes K_j/V_j for all keys of its batch (duplicated across the 2 cores of
a batch — cheap), and runs all 9 (i,j) attention pairs restricted to its half
of the query rows. Attention output for a query is invariant to key order, so
per-core inputs are column-permuted to put the core's query half first,
letting a single NEFF serve all 8 cores with no collectives.

Layouts (per core):
  X_i   = feat_i[b] as [C=256, N_i]  (channel-major, the native feat layout)
  Q^T_i = qw_i @ X_i[:, :NQ_i]       [256, NQ_i]
  K^T_j = kw_j @ X_j                 [256, N_j]
  V_j   = X_j^T @ vw_j^T (+ ones col)[N_j, 257]
  scores^T = (K^T)^T-tile @ Q^T      [keys, q]   (PE, f32r)
  E = exp(scores * SCALE)            (ACT, no max-subtraction needed: |s|<~2)
  AV: E_slice @ V_hat -> [q, 257]    (col 256 = softmax denominator, free)
  out-proj + residual + LayerNorm in [rows, 256] space, PE-transpose to
  channel-major for the output.
"""

import sys

for _p in ("/opt/trn_rl_repo",):
    if _p not in sys.path:
        sys.path.insert(0, _p)

import numpy as np
from contextlib import ExitStack

import concourse.bass as bass
import concourse.tile as tile
from concourse import bacc
from concourse import mybir
from concourse.masks import make_identity

P = 128
D = 256
S = 3
HW = [64, 32, 16]
NF = [4096, 1024, 256]          # full sequence lengths per scale
NQ = [2048, 512, 128]           # per-core query rows (half of NF)
B = 4
SCALE = float((D // 8) ** -0.5)
EPS = 1e-5
CK = 512                        # q-chunk width
F32 = mybir.dt.float32
F32R = mybir.dt.float32r
AF = mybir.ActivationFunctionType
ALU = mybir.AluOpType


def _r(ap):
    """View an fp32 AP as float32r for full-rate PE matmuls."""
    return ap.bitcast(F32R)


def _bcast(ap1d, p=P):
    """Partition-broadcast AP (stride-0 partition dim) for DMA replication."""
    return bass.AP(
        tensor=ap1d.tensor,
        offset=ap1d.offset,
        ap=[[0, p]] + [list(x) for x in ap1d.ap],
    )


def build_program(repeat=1):
    nc = bacc.Bacc("TRN2", target_bir_lowering=False, debug=False)

    x = [
        nc.dram_tensor(f"x{i}", [D, NF[i]], F32, kind="ExternalInput").ap()
        for i in range(S)
    ]
    w_dram = {}
    for nm in ("qw", "kw", "vw", "ow"):
        w_dram[nm] = nc.dram_tensor(nm, [S, D, D], F32, kind="ExternalInput").ap()
    b_dram = {}
    for nm in ("qb", "kb", "vb", "ob", "ln_g", "ln_b"):
        b_dram[nm] = nc.dram_tensor(nm, [S, D], F32, kind="ExternalInput").ap()
    y = [
        nc.dram_tensor(f"y{i}", [D, NQ[i]], F32, kind="ExternalOutput").ap()
        for i in range(S)
    ]

    with tile.TileContext(nc) as tc:
        if repeat == 1:
            with ExitStack() as ctx:
                _emit(tc, ctx, x, w_dram, b_dram, y)
        else:
            # hardware loop: repeat the whole computation inside one NEFF so
            # on-device time dominates the ~80 ms axon dispatch overhead
            with tc.For_i(0, repeat, 1):
                with ExitStack() as ctx:
                    _emit(tc, ctx, x, w_dram, b_dram, y)
    nc.compile()
    return nc


def _emit(tc, ctx, x, w_dram, b_dram, y):
    nc = tc.nc

    singles = ctx.enter_context(tc.tile_pool(name="singles", bufs=1))
    tpsum = ctx.enter_context(tc.tile_pool(name="tpsum", bufs=2, space="PSUM"))
    spsum = ctx.enter_context(tc.tile_pool(name="spsum", bufs=2, space="PSUM"))
    apsum = ctx.enter_context(tc.tile_pool(name="apsum", bufs=4, space="PSUM"))

    ident = singles.tile([P, P], F32, tag="ident")
    make_identity(nc, ident)

    eps_t = singles.tile([P, 1], F32, tag="eps")
    nc.vector.memset(eps_t, EPS)

    # ---- biases ----
    # per-partition layout (value indexed by output-channel on partitions)
    bk = {}
    for nm in ("qb", "kb", "ob", "ln_g", "ln_b"):
        for s in range(S):
            t = singles.tile([P, 2], F32, tag=f"bk_{nm}{s}")
            nc.gpsimd.dma_start(
                out=t, in_=b_dram[nm][s].rearrange("(po pi) -> pi po", pi=P)
            )
            bk[(nm, s)] = t
    # partition-broadcast layout (value indexed along the free dim)
    bb = {}
    for nm in ("vb", "ob", "ln_g", "ln_b"):
        for s in range(S):
            t = singles.tile([P, D], F32, tag=f"bb_{nm}{s}")
            nc.gpsimd.dma_start(out=t, in_=_bcast(b_dram[nm][s]))
            bb[(nm, s)] = t

    # ---- transposed weights:  wT[(kind,s)] is [Din_pi, Din_po, Dout] ----
    with tc.tile_pool(name="wstage", bufs=2) as wstage, \
            tc.tile_pool(name="xstage", bufs=1) as xstage:
        wT = {}
        for kind in ("qw", "kw", "vw", "ow"):
            for s in range(S):
                wsb = wstage.tile([P, 2, D], F32, tag="wload")
                nc.sync.dma_start(
                    out=wsb,
                    in_=w_dram[kind][s].rearrange("(po pi) f -> pi po f", pi=P),
                )
                wt = singles.tile([P, 2, D], F32R, tag=f"wT_{kind}{s}")
                for a in range(2):       # Din outer
                    for bo in range(2):  # Dout outer
                        pt = tpsum.tile([P, 512], F32, tag="t", name="t")[:, :P]
                        nc.tensor.transpose(pt, wsb[:, bo, a * P:(a + 1) * P], ident)
                        if kind == "ow":
                            # fold the "combined / S" into the output proj
                            nc.vector.tensor_scalar_mul(
                                out=wt[:, a, bo * P:(bo + 1) * P], in0=pt,
                                scalar1=1.0 / S,
                            )
                        else:
                            nc.vector.tensor_copy(
                                out=wt[:, a, bo * P:(bo + 1) * P], in_=pt
                            )
                wT[(kind, s)] = wt

        # ---- projections ----
        kT, vh, qT = {}, {}, {}
        for s in range(S):
            # DMA X in chunks and round each into a persistent f32r tile:
            # the BIR verifier requires every writer of an f32r-matmul operand
            # location to be a compute op with f32r output (DMA can't round).
            xs = xstage.tile([P, 2, NF[s]], F32R, tag="x")
            for c0 in range(0, NF[s], CK):
                w = min(CK, NF[s] - c0)
                xld = wstage.tile([P, 2, CK], F32, tag="xld", name="xld")[:, :, :w]
                nc.sync.dma_start(
                    out=xld,
                    in_=x[s].rearrange("(po pi) n -> pi po n", pi=P)[:, :, c0:c0 + w],
                )
                nc.vector.tensor_copy(out=xs[:, :, c0:c0 + w], in_=xld)

            # K^T_s : [Dout_pi, Dout_po, N]
            kt = singles.tile([P, 2, NF[s]], F32R, tag=f"kT{s}")
            for dt_ in range(2):
                for c0 in range(0, NF[s], CK):
                    w = min(CK, NF[s] - c0)
                    ps = tpsum.tile([P, 512], F32, tag="t", name="t")[:, :w]
                    for a in range(2):
                        nc.tensor.matmul(
                            ps,
                            lhsT=wT[("kw", s)][:, a, dt_ * P:(dt_ + 1) * P],
                            rhs=_r(xs[:, a, c0:c0 + w]),
                            start=(a == 0),
                            stop=(a == 1),
                        )
                    nc.vector.tensor_scalar_add(
                        out=kt[:, dt_, c0:c0 + w],
                        in0=ps,
                        scalar1=bk[("kb", s)][:, dt_:dt_ + 1],
                    )
            kT[s] = kt

            # V_s padded to 258 cols: col 256 = ones (softmax denominator
            # accumulator), col 257 = zeros (fp32r matmuls need an even
            # innermost dst count, and f32r Memset is not a valid ISA inst,
            # so both extras are written via tensor_scalar).
            vt = singles.tile([P, NF[s] // P, D + 2], F32R, tag=f"v{s}")
            for rt in range(NF[s] // P):
                ps = tpsum.tile([P, 512], F32, tag="t", name="t")[:, :D]
                for a in range(2):
                    nc.tensor.matmul(
                        ps,
                        lhsT=_r(xs[:, a, rt * P:(rt + 1) * P]),
                        rhs=wT[("vw", s)][:, a, :],
                        start=(a == 0),
                        stop=(a == 1),
                    )
                nc.vector.tensor_add(
                    out=vt[:, rt, :D], in0=ps, in1=bb[("vb", s)]
                )
                nc.vector.tensor_scalar(
                    out=vt[:, rt, D:D + 1], in0=eps_t,
                    scalar1=0.0, scalar2=1.0, op0=ALU.mult, op1=ALU.add,
                )
                nc.vector.tensor_scalar(
                    out=vt[:, rt, D + 1:D + 2], in0=eps_t,
                    scalar1=0.0, scalar2=None, op0=ALU.mult,
                )
            vh[s] = vt

            # Q^T_s : [Dout_pi, Dout_po, NQ]
            qt = singles.tile([P, 2, NQ[s]], F32R, tag=f"qT{s}")
            for dt_ in range(2):
                for c0 in range(0, NQ[s], CK):
                    w = min(CK, NQ[s] - c0)
                    ps = tpsum.tile([P, 512], F32, tag="t", name="t")[:, :w]
                    for a in range(2):
                        nc.tensor.matmul(
                            ps,
                            lhsT=wT[("qw", s)][:, a, dt_ * P:(dt_ + 1) * P],
                            rhs=_r(xs[:, a, c0:c0 + w]),
                            start=(a == 0),
                            stop=(a == 1),
                        )
                    nc.vector.tensor_scalar_add(
                        out=qt[:, dt_, c0:c0 + w],
                        in0=ps,
                        scalar1=bk[("qb", s)][:, dt_:dt_ + 1],
                    )
            qT[s] = qt

    # ---- attention + output ----
    etp = ctx.enter_context(tc.tile_pool(name="etp", bufs=3))
    combp = ctx.enter_context(tc.tile_pool(name="combp", bufs=2))
    outp = ctx.enter_context(tc.tile_pool(name="outp", bufs=2))
    miscp = ctx.enter_context(tc.tile_pool(name="miscp", bufs=4))

    for i in range(S):
        W = min(CK, NQ[i])
        for c0 in range(0, NQ[i], W):
            nqs = W // P
            comb = combp.tile([P, 4, D], F32, tag="comb", name="comb")[:, :nqs, :]

            for j in range(S):
                ktiles = NF[j] // P
                avp = [
                    apsum.tile([P, 512], F32, tag="av", name="av")[:, :D + 2]
                    for _ in range(nqs)
                ]
                for kt_ in range(ktiles):
                    sps = spsum.tile([P, 512], F32, tag="s", name="s")[:, :W]
                    for a in range(2):
                        nc.tensor.matmul(
                            sps,
                            lhsT=kT[j][:, a, kt_ * P:(kt_ + 1) * P],
                            rhs=qT[i][:, a, c0:c0 + W],
                            start=(a == 0),
                            stop=(a == 1),
                        )
                    et = etp.tile([P, 512], F32R, tag="et", name="et")[:, :W]
                    nc.scalar.activation(out=et, in_=sps, func=AF.Exp, scale=SCALE)
                    for qs in range(nqs):
                        nc.tensor.matmul(
                            avp[qs],
                            lhsT=et[:, qs * P:(qs + 1) * P],
                            rhs=vh[j][:, kt_, :],
                            start=(kt_ == 0),
                            stop=(kt_ == ktiles - 1),
                        )
                for qs in range(nqs):
                    rec = miscp.tile([P, 1], F32, tag="rec")
                    nc.vector.reciprocal(out=rec, in_=avp[qs][:, D:D + 1])
                    if j == 0:
                        nc.vector.tensor_scalar_mul(
                            out=comb[:, qs, :], in0=avp[qs][:, :D], scalar1=rec
                        )
                    else:
                        tmp = miscp.tile([P, D], F32, tag="ntmp")
                        nc.vector.tensor_scalar_mul(
                            out=tmp, in0=avp[qs][:, :D], scalar1=rec
                        )
                        nc.vector.tensor_add(
                            out=comb[:, qs, :], in0=comb[:, qs, :], in1=tmp
                        )

            # ---- output projection + residual + LN for this chunk ----
            cT = outp.tile([P, 2, CK], F32R, tag="cT", name="cT")[:, :, :W]
            for qs in range(nqs):
                for a in range(2):
                    pt = tpsum.tile([P, 512], F32, tag="t", name="t")[:, :P]
                    nc.tensor.transpose(pt, comb[:, qs, a * P:(a + 1) * P], ident)
                    nc.vector.tensor_copy(
                        out=cT[:, a, qs * P:(qs + 1) * P], in_=pt
                    )
            xr = outp.tile([P, 2, CK], F32, tag="xr", name="xr")[:, :, :W]
            nc.sync.dma_start(
                out=xr,
                in_=x[i].rearrange("(po pi) n -> pi po n", pi=P)[:, :, c0:c0 + W],
            )
            ysb = outp.tile([P, 2, CK], F32, tag="ysb", name="ysb")[:, :, :W]
            for qs in range(nqs):
                ops = tpsum.tile([P, 512], F32, tag="t", name="t")[:, :D]
                for a in range(2):
                    nc.tensor.matmul(
                        ops,
                        lhsT=cT[:, a, qs * P:(qs + 1) * P],
                        rhs=wT[("ow", i)][:, a, :],
                        start=(a == 0),
                        stop=(a == 1),
                    )
                o = miscp.tile([P, D], F32, tag="o")
                nc.vector.tensor_add(out=o, in0=ops, in1=bb[("ob", i)])
                for a in range(2):
                    xt = tpsum.tile([P, 512], F32, tag="t", name="t")[:, :P]
                    nc.tensor.transpose(xt, xr[:, a, qs * P:(qs + 1) * P], ident)
                    nc.vector.tensor_add(
                        out=o[:, a * P:(a + 1) * P],
                        in0=o[:, a * P:(a + 1) * P],
                        in1=xt,
                    )
                # LayerNorm over the free dim
                stats = miscp.tile([P, 6], F32, tag="st")
                nc.vector.bn_stats(out=stats, in_=o)
                mv = miscp.tile([P, 2], F32, tag="mv")
                nc.vector.bn_aggr(out=mv, in_=stats)
                nc.scalar.activation(
                    out=mv[:, 1:2], in_=mv[:, 1:2], func=AF.Sqrt,
                    bias=eps_t, scale=1.0,
                )
                nc.vector.reciprocal(out=mv[:, 1:2], in_=mv[:, 1:2])
                nc.vector.tensor_scalar(
                    out=o, in0=o,
                    scalar1=mv[:, 0:1], scalar2=mv[:, 1:2],
                    op0=ALU.subtract, op1=ALU.mult,
                )
                nc.vector.tensor_mul(out=o, in0=o, in1=bb[("ln_g", i)])
                nc.vector.tensor_add(out=o, in0=o, in1=bb[("ln_b", i)])
                for a in range(2):
                    yt = tpsum.tile([P, 512], F32, tag="t", name="t")[:, :P]
                    nc.tensor.transpose(yt, o[:, a * P:(a + 1) * P], ident)
                    nc.vector.tensor_copy(
                        out=ysb[:, a, qs * P:(qs + 1) * P], in_=yt
                    )
            nc.sync.dma_start(
                out=y[i].rearrange("(po pi) n -> pi po n", pi=P)[:, :, c0:c0 + W],
                in_=ysb,
            )


# ---------------------------------------------------------------------------
# host-side sharding / execution
# ---------------------------------------------------------------------------

_CACHE = {}


def _shard_inputs(inputs):
    feats = [np.ascontiguousarray(np.asarray(inputs[f"feat{i}"], np.float32))
             for i in range(S)]
    consts = {
        nm: np.ascontiguousarray(np.asarray(inputs[nm], np.float32))
        for nm in ("qw", "kw", "vw", "ow", "qb", "kb", "vb", "ob", "ln_g", "ln_b")
    }
    in_maps = []
    for c in range(8):
        b, h = c // 2, c % 2
        m = dict(consts)
        for i in range(S):
            xi = feats[i][b].reshape(D, NF[i])
            if h == 1:
                xi = np.concatenate([xi[:, NF[i] // 2:], xi[:, :NF[i] // 2]], axis=1)
            m[f"x{i}"] = np.ascontiguousarray(xi)
        in_maps.append(m)
    return in_maps


def _unshard_outputs(results):
    outs = []
    for i in range(S):
        full = np.empty((B, D, NF[i]), np.float32)
        for c in range(8):
            b, h = c // 2, c % 2
            full[b][:, h * NQ[i]:(h + 1) * NQ[i]] = results[c][f"y{i}"]
        outs.append(full.reshape(B, D, HW[i], HW[i]))
    return tuple(outs)


def get_nc():
    if "nc" not in _CACHE:
        _CACHE["nc"] = build_program()
    return _CACHE["nc"]


def _get_runner():
    if "runner" not in _CACHE:
        _CACHE["runner"] = _build_runner(get_nc())
    return _CACHE["runner"]


def _build_runner(nc):
    """Build a jitted 8-core SPMD executor for a Bass program.

    Mirrors bass2jax.run_bass_via_pjrt's multi-core path, but caches the
    jitted callable so repeat calls don't re-trace/re-compile. No donation:
    the kernel writes every output element, so fresh result buffers are fine.
    """
    import jax
    from jax.sharding import Mesh, PartitionSpec
    from jax.experimental.shard_map import shard_map
    from concourse import bass2jax, mybir

    bass2jax.install_neuronx_cc_hook()

    partition_name = (
        nc.partition_id_tensor.name if nc.partition_id_tensor else None
    )
    in_names, out_names, out_avals, zero_outs = [], [], [], []
    for alloc in nc.m.functions[0].allocations:
        if not isinstance(alloc, mybir.MemoryLocationSet):
            continue
        name = alloc.memorylocations[0].name
        if alloc.kind == "ExternalInput":
            if name != partition_name:
                in_names.append(name)
        elif alloc.kind == "ExternalOutput":
            shape = tuple(alloc.tensor_shape)
            dtype = mybir.dt.np(alloc.dtype)
            out_names.append(name)
            out_avals.append(jax.core.ShapedArray(shape, dtype))
            zero_outs.append(np.zeros(shape, dtype))
    n_params = len(in_names)
    all_names = in_names + out_names
    if partition_name is not None:
        all_names = all_names + [partition_name]

    def _body(*args):
        operands = list(args)
        if partition_name is not None:
            operands.append(bass2jax.partition_id_tensor())
        outs = bass2jax._bass_exec_p.bind(
            *operands,
            out_avals=tuple(out_avals),
            in_names=tuple(all_names),
            out_names=tuple(out_names),
            lowering_input_output_aliases=(),
            sim_require_finite=True,
            sim_require_nnan=True,
            nc=nc,
        )
        return tuple(outs)

    n_cores = 8
    devices = jax.devices()[:n_cores]
    mesh = Mesh(np.asarray(devices), ("core",))
    specs = (PartitionSpec("core"),) * (n_params + len(out_names))
    sharded = jax.jit(
        shard_map(
            _body, mesh=mesh,
            in_specs=specs,
            out_specs=(PartitionSpec("core"),) * len(out_names),
            check_rep=False,
        ),
        keep_unused=True,
    )

    concat_zeros = [
        np.zeros((n_cores * z.shape[0], *z.shape[1:]), z.dtype) for z in zero_outs
    ]
    zeros_dev = [jax.device_put(z, jax.NamedSharding(mesh, PartitionSpec("core")))
                 for z in concat_zeros]

    def prepare(in_maps):
        assert len(in_maps) == n_cores
        concat_in = [
            np.concatenate([np.asarray(in_maps[c][nm]) for c in range(n_cores)],
                           axis=0)
            for nm in in_names
        ]
        return [
            jax.device_put(a, jax.NamedSharding(mesh, PartitionSpec("core")))
            for a in concat_in
        ]

    def run(dev_in):
        out_arrs = sharded(*dev_in, *zeros_dev)
        jax.block_until_ready(out_arrs)
        return out_arrs

    def unpack(out_arrs):
        return [
            {
                nm: np.asarray(out_arrs[i]).reshape(n_cores, *out_avals[i].shape)[c]
                for i, nm in enumerate(out_names)
            }
            for c in range(n_cores)
        ]

    return (prepare, run, unpack)


def kernel(**inputs):
    prepare, run, unpack = _get_runner()
    in_maps = _shard_inputs(inputs)
    results = unpack(run(prepare(in_maps)))
    return _unshard_outputs(results)


if __name__ == "__main__":
    nc = build_program()
    print("program built ok")


def measure_hw_ns(inputs, repeat=64, samples=20):
    """Measure on-device kernel time by differencing an R-repeat hardware
    loop build against the single-shot build (cancels the ~80 ms axon
    dispatch overhead)."""
    import time as _time
    import jax

    in_maps = _shard_inputs(inputs)

    def best_wall(nc_runner):
        prepare, run, unpack = nc_runner
        dev_in = prepare(in_maps)
        run(dev_in)  # warmup/compile
        best = float("inf")
        for _ in range(samples):
            t0 = _time.perf_counter()
            run(dev_in)
            best = min(best, _time.perf_counter() - t0)
        return best, (prepare, run, unpack), dev_in

    t1, _, _ = best_wall(_get_runner())
    if "rep_runner" not in _CACHE:
        _CACHE["rep_runner"] = _build_runner(build_program(repeat=repeat))
    tR, runner, dev_in = best_wall(_CACHE["rep_runner"])
    per_iter = (tR - t1) / (repeat - 1)
    outs = runner[2](runner[1](dev_in))
    return per_iter * 1e9, t1, tR, outs


# revision 16
# speedup vs baseline: 3661.9893x; 1.1564x over previous
"""Cross-scale attention kernel for Trainium2 (8 NeuronCores, SPMD).

Sharding: core c handles batch b = c//2 and query-half h = c%2. Each core
computes K_j/V_j for all keys of its batch (duplicated across the 2 cores of
a batch — cheap), and runs all 9 (i,j) attention pairs restricted to its half
of the query rows. Attention output for a query is invariant to key order, so
per-core inputs are column-permuted to put the core's query half first,
letting a single NEFF serve all 8 cores with no collectives.

Layouts (per core):
  X_i   = feat_i[b] as [C=256, N_i]  (channel-major, the native feat layout)
  Q^T_i = qw_i @ X_i[:, :NQ_i]       [256, NQ_i]
  K^T_j = kw_j @ X_j                 [256, N_j]
  V_j   = X_j^T @ vw_j^T (+ ones col)[N_j, 257]
  scores^T = (K^T)^T-tile @ Q^T      [keys, q]   (PE, f32r)
  E = exp(scores * SCALE)            (ACT, no max-subtraction needed: |s|<~2)
  AV: E_slice @ V_hat -> [q, 257]    (col 256 = softmax denominator, free)
  out-proj + residual + LayerNorm in [rows, 256] space, PE-transpose to
  channel-major for the output.
"""

import sys

for _p in ("/opt/trn_rl_repo",):
    if _p not in sys.path:
        sys.path.insert(0, _p)

import numpy as np
from contextlib import ExitStack

import concourse.bass as bass
import concourse.tile as tile
from concourse import bacc
from concourse import mybir
from concourse.masks import make_identity

P = 128
D = 256
S = 3
HW = [64, 32, 16]
NF = [4096, 1024, 256]          # full sequence lengths per scale
NQ = [2048, 512, 128]           # per-core query rows (half of NF)
B = 4
SCALE = float((D // 8) ** -0.5)
EPS = 1e-5
CK = 512                        # q-chunk width
F32 = mybir.dt.float32
F32R = mybir.dt.float32r
AF = mybir.ActivationFunctionType
ALU = mybir.AluOpType


def _r(ap):
    """View an fp32 AP as float32r for full-rate PE matmuls."""
    return ap.bitcast(F32R)


def _bcast(ap1d, p=P):
    """Partition-broadcast AP (stride-0 partition dim) for DMA replication."""
    return bass.AP(
        tensor=ap1d.tensor,
        offset=ap1d.offset,
        ap=[[0, p]] + [list(x) for x in ap1d.ap],
    )


class _Bacc(bacc.Bacc):
    """Bacc whose ACT-table chooser pins Exp and Ln to the one table set
    that contains both (natural_log_exp_and_others), so the kernel needs a
    single ACT_TABLE_LOAD instead of thrashing ~2.7us reloads between the
    softmax Exps and the LayerNorm rsqrt Ln/Exp pair."""

    def insert_act_table_loads(self):
        from concourse.hw_specs import get_activation_tables
        from concourse.bacc import _bass_rust as _br

        has_activation = any(
            isinstance(i, mybir.InstActivation)
            for b in self.main_func.blocks
            for i in b.instructions
        )
        if not has_activation:
            return
        strip = {AF.Exp, AF.Ln}
        tables = [
            (name, funcs if name == "natural_log_exp_and_others"
             else funcs - strip)
            for name, funcs in get_activation_tables(self.m.arch).items()
        ]
        _br.insert_act_table_loads(self, tables)


def build_program(repeat=1):
    nc = _Bacc("TRN2", target_bir_lowering=False, debug=False)

    x = [
        nc.dram_tensor(f"x{i}", [D, NF[i]], F32, kind="ExternalInput").ap()
        for i in range(S)
    ]
    w_dram = {}
    for nm in ("qw", "kw", "vw", "ow"):
        w_dram[nm] = nc.dram_tensor(nm, [S, D, D], F32, kind="ExternalInput").ap()
    b_dram = {}
    for nm in ("qb", "kb", "vb", "ob", "ln_g", "ln_b"):
        b_dram[nm] = nc.dram_tensor(nm, [S, D], F32, kind="ExternalInput").ap()
    y = [
        nc.dram_tensor(f"y{i}", [D, NQ[i]], F32, kind="ExternalOutput").ap()
        for i in range(S)
    ]

    with tile.TileContext(nc) as tc:
        if repeat == 1:
            with ExitStack() as ctx:
                _emit(tc, ctx, x, w_dram, b_dram, y)
        else:
            # hardware loop: repeat the whole computation inside one NEFF so
            # on-device time dominates the ~80 ms axon dispatch overhead
            with tc.For_i(0, repeat, 1):
                with ExitStack() as ctx:
                    _emit(tc, ctx, x, w_dram, b_dram, y)
    nc.compile()
    return nc


def _emit(tc, ctx, x, w_dram, b_dram, y):
    nc = tc.nc

    singles = ctx.enter_context(tc.tile_pool(name="singles", bufs=1))
    tpsum = ctx.enter_context(tc.tile_pool(name="tpsum", bufs=2, space="PSUM"))
    spsum = ctx.enter_context(tc.tile_pool(name="spsum", bufs=2, space="PSUM"))
    apsum = ctx.enter_context(tc.tile_pool(name="apsum", bufs=4, space="PSUM"))

    ident = singles.tile([P, P], F32, tag="ident")
    make_identity(nc, ident)

    eps_t = singles.tile([P, 1], F32, tag="eps")
    nc.vector.memset(eps_t, EPS)

    # ---- biases ----
    bk = {}
    for nm in ("qb", "kb"):
        for s in range(S):
            t = singles.tile([P, 2], F32, tag=f"bk_{nm}{s}")
            nc.gpsimd.dma_start(
                out=t, in_=b_dram[nm][s].rearrange("(po pi) -> pi po", pi=P)
            )
            bk[(nm, s)] = t
    bb = {}
    for nm in ("vb", "ob", "ln_g", "ln_b"):
        for s in range(S):
            t = singles.tile([P, D], F32, tag=f"bb_{nm}{s}")
            nc.gpsimd.dma_start(out=t, in_=_bcast(b_dram[nm][s]))
            bb[(nm, s)] = t

    # ---- transposed weights:  wT[(kind,s)] is [Din_pi, Din_po, Dout] ----
    KINDS = [("qw", s) for s in range(S)] + [("kw", s) for s in range(S)] + \
            [("vw", s) for s in range(S)] + [("ow", s) for s in range(S)]
    with tc.tile_pool(name="wstage", bufs=6) as wstage, \
            tc.tile_pool(name="xldstage", bufs=2) as xldstage, \
            tc.tile_pool(name="xstage", bufs=1) as xstage:
        # queue all weight DMAs up front so they pipeline through the DMA
        # engines while the PE chews transposes
        wsb = {}
        for kind, s in KINDS:
            t = wstage.tile([P, 2, D], F32, tag="wload", name="wload")
            nc.sync.dma_start(
                out=t, in_=w_dram[kind][s].rearrange("(po pi) f -> pi po f", pi=P)
            )
            wsb[(kind, s)] = t
        wT = {}
        for kind, s in KINDS:
            wt = singles.tile([P, 2, D], F32R, tag=f"wT_{kind}{s}")
            for a in range(2):       # Din outer
                for bo in range(2):  # Dout outer
                    pt = tpsum.tile([P, 512], F32, tag="t", name="t")[:, :P]
                    nc.tensor.transpose(
                        pt, wsb[(kind, s)][:, bo, a * P:(a + 1) * P], ident
                    )
                    if kind == "ow":
                        # fold the "combined / S" into the output proj
                        nc.vector.tensor_scalar_mul(
                            out=wt[:, a, bo * P:(bo + 1) * P], in0=pt,
                            scalar1=1.0 / S,
                        )
                    else:
                        nc.vector.tensor_copy(
                            out=wt[:, a, bo * P:(bo + 1) * P], in_=pt
                        )
            wT[(kind, s)] = wt

        # ---- projections ----
        # queries of all scales are packed into one global column space so
        # every attention chunk is >=256 wide (full-rate f32r matmuls)
        QOFF = [0, NQ[0], NQ[0] + NQ[1]]           # [0, 2048, 2560]
        NQT = NQ[0] + NQ[1] + NQ[2]                # 2688
        qTcat = singles.tile([P, 2, NQT], F32R, tag="qTcat")

        kT, vh = {}, {}
        for s in range(S):
            # DMA X in chunks, rounding each into a persistent f32r tile
            # (f32r matmul operands need a compute-engine producer)
            xs = xstage.tile([P, 2, NF[s]], F32R, tag="x")
            for c0 in range(0, NF[s], CK):
                w = min(CK, NF[s] - c0)
                xld = xldstage.tile([P, 2, CK], F32, tag="xld", name="xld")[:, :, :w]
                nc.sync.dma_start(
                    out=xld,
                    in_=x[s].rearrange("(po pi) n -> pi po n", pi=P)[:, :, c0:c0 + w],
                )
                nc.vector.tensor_copy(out=xs[:, :, c0:c0 + w], in_=xld)

            # K^T_s : [Dout_pi, Dout_po, N]
            kt = singles.tile([P, 2, NF[s]], F32R, tag=f"kT{s}")
            for dt_ in range(2):
                for c0 in range(0, NF[s], CK):
                    w = min(CK, NF[s] - c0)
                    ps = tpsum.tile([P, 512], F32, tag="t", name="t")[:, :w]
                    for a in range(2):
                        nc.tensor.matmul(
                            ps,
                            lhsT=wT[("kw", s)][:, a, dt_ * P:(dt_ + 1) * P],
                            rhs=_r(xs[:, a, c0:c0 + w]),
                            start=(a == 0),
                            stop=(a == 1),
                        )
                    nc.vector.tensor_scalar_add(
                        out=kt[:, dt_, c0:c0 + w],
                        in0=ps,
                        scalar1=bk[("kb", s)][:, dt_:dt_ + 1],
                    )
            kT[s] = kt

            # V_s padded to 258 cols: col 256 = ones (softmax denominator
            # accumulator), col 257 = zeros (fp32r matmuls need an even
            # innermost dst count; f32r Memset is not a valid ISA inst)
            vt = singles.tile([P, NF[s] // P, D + 2], F32R, tag=f"v{s}")
            for rt in range(NF[s] // P):
                ps = tpsum.tile([P, 512], F32, tag="t", name="t")[:, :D]
                for a in range(2):
                    nc.tensor.matmul(
                        ps,
                        lhsT=_r(xs[:, a, rt * P:(rt + 1) * P]),
                        rhs=wT[("vw", s)][:, a, :],
                        start=(a == 0),
                        stop=(a == 1),
                    )
                nc.vector.tensor_add(
                    out=vt[:, rt, :D], in0=ps, in1=bb[("vb", s)]
                )
                nc.vector.tensor_scalar(
                    out=vt[:, rt, D:D + 1], in0=eps_t,
                    scalar1=0.0, scalar2=1.0, op0=ALU.mult, op1=ALU.add,
                )
                nc.vector.tensor_scalar(
                    out=vt[:, rt, D + 1:D + 2], in0=eps_t,
                    scalar1=0.0, scalar2=None, op0=ALU.mult,
                )
            vh[s] = vt

            # Q^T_s into the packed query tile
            for dt_ in range(2):
                for c0 in range(0, NQ[s], CK):
                    w = min(CK, NQ[s] - c0)
                    ps = tpsum.tile([P, 512], F32, tag="t", name="t")[:, :w]
                    for a in range(2):
                        nc.tensor.matmul(
                            ps,
                            lhsT=wT[("qw", s)][:, a, dt_ * P:(dt_ + 1) * P],
                            rhs=_r(xs[:, a, c0:c0 + w]),
                            start=(a == 0),
                            stop=(a == 1),
                        )
                    nc.vector.tensor_scalar_add(
                        out=qTcat[:, dt_, QOFF[s] + c0:QOFF[s] + c0 + w],
                        in0=ps,
                        scalar1=bk[("qb", s)][:, dt_:dt_ + 1],
                    )

    # ---- attention over the packed query space ----
    etp = ctx.enter_context(tc.tile_pool(name="etp", bufs=3))
    combp = ctx.enter_context(tc.tile_pool(name="combp", bufs=3))
    ctp = ctx.enter_context(tc.tile_pool(name="ctp", bufs=6))
    op_ = ctx.enter_context(tc.tile_pool(name="op", bufs=8))
    xrp = ctx.enter_context(tc.tile_pool(name="xrp", bufs=6))
    ysbp = ctx.enter_context(tc.tile_pool(name="ysbp", bufs=4))
    miscp = ctx.enter_context(tc.tile_pool(name="miscp", bufs=8))

    QOFF = [0, NQ[0], NQ[0] + NQ[1]]
    CHUNKS = [(0, 512), (512, 512), (1024, 512), (1536, 512),
              (2048, 384), (2432, 256)]

    def sub_scale(g0):
        """global col -> (scale, local col)"""
        for s in range(S - 1, -1, -1):
            if g0 >= QOFF[s]:
                return s, g0 - QOFF[s]
        raise AssertionError

    pending = []

    def drain(k):
        for _ in range(min(k, len(pending))):
            pending.pop(0)()

    def stage_a(comb, qs, g0):
        def emit():
            s, lc0 = sub_scale(g0)
            cT = ctp.tile([P, 2, P], F32R, tag="cT", name="cT")
            for a in range(2):
                pt = tpsum.tile([P, 512], F32, tag="t", name="t")[:, :P]
                nc.tensor.transpose(pt, comb[:, qs, a * P:(a + 1) * P], ident)
                nc.vector.tensor_copy(out=cT[:, a, :], in_=pt)
            xr = xrp.tile([P, 2, P], F32, tag="xr", name="xr")
            nc.sync.dma_start(
                out=xr,
                in_=x[s].rearrange("(po pi) n -> pi po n", pi=P)[:, :, lc0:lc0 + P],
            )
            ops = tpsum.tile([P, 512], F32, tag="t", name="t")[:, :D]
            for a in range(2):
                nc.tensor.matmul(
                    ops,
                    lhsT=cT[:, a, :],
                    rhs=wT[("ow", s)][:, a, :],
                    start=(a == 0),
                    stop=(a == 1),
                )
            o = op_.tile([P, D], F32, tag="o", name="o")
            nc.vector.tensor_add(out=o, in0=ops, in1=bb[("ob", s)])
            for a in range(2):
                xt = tpsum.tile([P, 512], F32, tag="t", name="t")[:, :P]
                nc.tensor.transpose(xt, xr[:, a, :], ident)
                nc.vector.tensor_add(
                    out=o[:, a * P:(a + 1) * P],
                    in0=o[:, a * P:(a + 1) * P],
                    in1=xt,
                )
            # LayerNorm over the free dim; rstd = exp(-0.5*ln(var+eps)) so
            # Ln/Exp share the softmax Exp's ACT table set (no reloads)
            stats = miscp.tile([P, 6], F32, tag="st", name="st")
            nc.vector.bn_stats(out=stats, in_=o)
            mv = miscp.tile([P, 2], F32, tag="mv", name="mv")
            nc.vector.bn_aggr(out=mv, in_=stats)
            nc.scalar.activation(
                out=mv[:, 1:2], in_=mv[:, 1:2], func=AF.Ln,
                bias=eps_t, scale=1.0,
            )
            nc.scalar.activation(
                out=mv[:, 1:2], in_=mv[:, 1:2], func=AF.Exp, scale=-0.5,
            )
            nc.vector.tensor_scalar(
                out=o, in0=o,
                scalar1=mv[:, 0:1], scalar2=mv[:, 1:2],
                op0=ALU.subtract, op1=ALU.mult,
            )
            nc.vector.tensor_mul(out=o, in0=o, in1=bb[("ln_g", s)])
            nc.vector.tensor_add(out=o, in0=o, in1=bb[("ln_b", s)])
            pending.append(stage_b(o, g0))
        return emit

    def stage_b(o, g0):
        def emit():
            s, lc0 = sub_scale(g0)
            ysb = ysbp.tile([P, 2, P], F32, tag="ysb", name="ysb")
            for a in range(2):
                yt = tpsum.tile([P, 512], F32, tag="t", name="t")[:, :P]
                nc.tensor.transpose(yt, o[:, a * P:(a + 1) * P], ident)
                nc.vector.tensor_copy(out=ysb[:, a, :], in_=yt)
            nc.sync.dma_start(
                out=y[s].rearrange("(po pi) n -> pi po n", pi=P)[:, :, lc0:lc0 + P],
                in_=ysb,
            )
        return emit

    for g0, W in CHUNKS:
        nqs = W // P
        comb = combp.tile([P, 4, D], F32, tag="comb", name="comb")[:, :nqs, :]
        for j in range(S):
            ktiles = NF[j] // P
            avp = [
                apsum.tile([P, 512], F32, tag="av", name="av")[:, :D + 2]
                for _ in range(nqs)
            ]
            for kt_ in range(ktiles):
                sps = spsum.tile([P, 512], F32, tag="s", name="s")[:, :W]
                for a in range(2):
                    nc.tensor.matmul(
                        sps,
                        lhsT=kT[j][:, a, kt_ * P:(kt_ + 1) * P],
                        rhs=qTcat[:, a, g0:g0 + W],
                        start=(a == 0),
                        stop=(a == 1),
                    )
                et = etp.tile([P, 512], F32R, tag="et", name="et")[:, :W]
                nc.scalar.activation(out=et, in_=sps, func=AF.Exp, scale=SCALE)
                for qs in range(nqs):
                    nc.tensor.matmul(
                        avp[qs],
                        lhsT=et[:, qs * P:(qs + 1) * P],
                        rhs=vh[j][:, kt_, :],
                        start=(kt_ == 0),
                        stop=(kt_ == ktiles - 1),
                    )
            for qs in range(nqs):
                rec = miscp.tile([P, 1], F32, tag="rec", name="rec")
                nc.vector.reciprocal(out=rec, in_=avp[qs][:, D:D + 1])
                if j == 0:
                    nc.vector.tensor_scalar_mul(
                        out=comb[:, qs, :], in0=avp[qs][:, :D], scalar1=rec
                    )
                else:
                    tmp = miscp.tile([P, D], F32, tag="ntmp", name="ntmp")
                    nc.vector.tensor_scalar_mul(
                        out=tmp, in0=avp[qs][:, :D], scalar1=rec
                    )
                    nc.vector.tensor_add(
                        out=comb[:, qs, :], in0=comb[:, qs, :], in1=tmp
                    )
            # overlap deferred out-proj work of the previous chunk with this
            # chunk's attention so the in-order PE never stalls on LN chains
            drain(3)
        for qs in range(nqs):
            pending.append(stage_a(comb, qs, g0 + qs * P))
    drain(len(pending) + 64)
    while pending:
        drain(len(pending) + 64)


# ---------------------------------------------------------------------------
# host-side sharding / execution
# ---------------------------------------------------------------------------

_CACHE = {}


def _shard_inputs(inputs):
    feats = [np.ascontiguousarray(np.asarray(inputs[f"feat{i}"], np.float32))
             for i in range(S)]
    consts = {
        nm: np.ascontiguousarray(np.asarray(inputs[nm], np.float32))
        for nm in ("qw", "kw", "vw", "ow", "qb", "kb", "vb", "ob", "ln_g", "ln_b")
    }
    in_maps = []
    for c in range(8):
        b, h = c // 2, c % 2
        m = dict(consts)
        for i in range(S):
            xi = feats[i][b].reshape(D, NF[i])
            if h == 1:
                xi = np.concatenate([xi[:, NF[i] // 2:], xi[:, :NF[i] // 2]], axis=1)
            m[f"x{i}"] = np.ascontiguousarray(xi)
        in_maps.append(m)
    return in_maps


def _unshard_outputs(results):
    outs = []
    for i in range(S):
        full = np.empty((B, D, NF[i]), np.float32)
        for c in range(8):
            b, h = c // 2, c % 2
            full[b][:, h * NQ[i]:(h + 1) * NQ[i]] = results[c][f"y{i}"]
        outs.append(full.reshape(B, D, HW[i], HW[i]))
    return tuple(outs)


def get_nc():
    if "nc" not in _CACHE:
        _CACHE["nc"] = build_program()
    return _CACHE["nc"]


def _get_runner():
    if "runner" not in _CACHE:
        _CACHE["runner"] = _build_runner(get_nc())
    return _CACHE["runner"]


def _build_runner(nc):
    """Build a jitted 8-core SPMD executor for a Bass program.

    Mirrors bass2jax.run_bass_via_pjrt's multi-core path, but caches the
    jitted callable so repeat calls don't re-trace/re-compile. No donation:
    the kernel writes every output element, so fresh result buffers are fine.
    """
    import jax
    from jax.sharding import Mesh, PartitionSpec
    from jax.experimental.shard_map import shard_map
    from concourse import bass2jax, mybir

    bass2jax.install_neuronx_cc_hook()

    partition_name = (
        nc.partition_id_tensor.name if nc.partition_id_tensor else None
    )
    in_names, out_names, out_avals, zero_outs = [], [], [], []
    for alloc in nc.m.functions[0].allocations:
        if not isinstance(alloc, mybir.MemoryLocationSet):
            continue
        name = alloc.memorylocations[0].name
        if alloc.kind == "ExternalInput":
            if name != partition_name:
                in_names.append(name)
        elif alloc.kind == "ExternalOutput":
            shape = tuple(alloc.tensor_shape)
            dtype = mybir.dt.np(alloc.dtype)
            out_names.append(name)
            out_avals.append(jax.core.ShapedArray(shape, dtype))
            zero_outs.append(np.zeros(shape, dtype))
    n_params = len(in_names)
    all_names = in_names + out_names
    if partition_name is not None:
        all_names = all_names + [partition_name]

    def _body(*args):
        operands = list(args)
        if partition_name is not None:
            operands.append(bass2jax.partition_id_tensor())
        outs = bass2jax._bass_exec_p.bind(
            *operands,
            out_avals=tuple(out_avals),
            in_names=tuple(all_names),
            out_names=tuple(out_names),
            lowering_input_output_aliases=(),
            sim_require_finite=True,
            sim_require_nnan=True,
            nc=nc,
        )
        return tuple(outs)

    n_cores = 8
    devices = jax.devices()[:n_cores]
    mesh = Mesh(np.asarray(devices), ("core",))
    specs = (PartitionSpec("core"),) * (n_params + len(out_names))
    sharded = jax.jit(
        shard_map(
            _body, mesh=mesh,
            in_specs=specs,
            out_specs=(PartitionSpec("core"),) * len(out_names),
            check_rep=False,
        ),
        keep_unused=True,
    )

    concat_zeros = [
        np.zeros((n_cores * z.shape[0], *z.shape[1:]), z.dtype) for z in zero_outs
    ]
    zeros_dev = [jax.device_put(z, jax.NamedSharding(mesh, PartitionSpec("core")))
                 for z in concat_zeros]

    def prepare(in_maps):
        assert len(in_maps) == n_cores
        concat_in = [
            np.concatenate([np.asarray(in_maps[c][nm]) for c in range(n_cores)],
                           axis=0)
            for nm in in_names
        ]
        return [
            jax.device_put(a, jax.NamedSharding(mesh, PartitionSpec("core")))
            for a in concat_in
        ]

    def run(dev_in):
        out_arrs = sharded(*dev_in, *zeros_dev)
        jax.block_until_ready(out_arrs)
        return out_arrs

    def unpack(out_arrs):
        return [
            {
                nm: np.asarray(out_arrs[i]).reshape(n_cores, *out_avals[i].shape)[c]
                for i, nm in enumerate(out_names)
            }
            for c in range(n_cores)
        ]

    return (prepare, run, unpack)


def kernel(**inputs):
    prepare, run, unpack = _get_runner()
    in_maps = _shard_inputs(inputs)
    results = unpack(run(prepare(in_maps)))
    return _unshard_outputs(results)


if __name__ == "__main__":
    nc = build_program()
    print("program built ok")


def measure_hw_ns(inputs, r_small=256, r_big=1024, samples=12):
    """Measure on-device kernel time by differencing two hardware-loop
    builds (r_big vs r_small iterations inside one NEFF). The ~80 ms axon
    dispatch overhead and any per-invocation constants cancel; samples are
    interleaved so tunnel-latency drift affects both builds equally."""
    import time as _time

    in_maps = _shard_inputs(inputs)

    def get_runner(r):
        key = ("rep", r)
        if key not in _CACHE:
            _CACHE[key] = _build_runner(build_program(repeat=r))
        return _CACHE[key]

    ra, rb = get_runner(r_small), get_runner(r_big)
    dev_a = ra[0](in_maps)
    dev_b = rb[0](in_maps)
    ra[1](dev_a)  # warmup / compile
    rb[1](dev_b)
    ta, tb = [], []
    for _ in range(samples):
        t0 = _time.perf_counter()
        ra[1](dev_a)
        ta.append(_time.perf_counter() - t0)
        t0 = _time.perf_counter()
        rb[1](dev_b)
        tb.append(_time.perf_counter() - t0)
    best_a, best_b = min(ta), min(tb)
    per_iter = (best_b - best_a) / (r_big - r_small)
    outs = rb[2](rb[1](dev_b))
    return per_iter * 1e9, best_a, best_b, outs


# revision 23
# speedup vs baseline: 3811.8702x; 1.0409x over previous
"""Cross-scale attention kernel for Trainium2 (8 NeuronCores, SPMD).

Sharding: core c handles batch b = c//2 and query-half h = c%2. Each core
computes K_j/V_j for all keys of its batch (duplicated across the 2 cores of
a batch — cheap), and runs all 9 (i,j) attention pairs restricted to its half
of the query rows. Attention output for a query is invariant to key order, so
per-core inputs are column-permuted to put the core's query half first,
letting a single NEFF serve all 8 cores with no collectives.

Layouts (per core):
  X_i   = feat_i[b] as [C=256, N_i]  (channel-major, the native feat layout)
  Q^T_i = qw_i @ X_i[:, :NQ_i]       [256, NQ_i]
  K^T_j = kw_j @ X_j                 [256, N_j]
  V_j   = X_j^T @ vw_j^T (+ ones col)[N_j, 257]
  scores^T = (K^T)^T-tile @ Q^T      [keys, q]   (PE, f32r)
  E = exp(scores * SCALE)            (ACT, no max-subtraction needed: |s|<~2)
  AV: E_slice @ V_hat -> [q, 257]    (col 256 = softmax denominator, free)
  out-proj + residual + LayerNorm in [rows, 256] space, PE-transpose to
  channel-major for the output.
"""

import sys

for _p in ("/opt/trn_rl_repo",):
    if _p not in sys.path:
        sys.path.insert(0, _p)

import numpy as np
from contextlib import ExitStack

import concourse.bass as bass
import concourse.tile as tile
from concourse import bacc
from concourse import mybir
from concourse.masks import make_identity

P = 128
D = 256
S = 3
HW = [64, 32, 16]
NF = [4096, 1024, 256]          # full sequence lengths per scale
NQ = [2048, 512, 128]           # per-core query rows (half of NF)
B = 4
SCALE = float((D // 8) ** -0.5)
EPS = 1e-5
CK = 512                        # q-chunk width
F32 = mybir.dt.float32
F32R = mybir.dt.float32r
AF = mybir.ActivationFunctionType
ALU = mybir.AluOpType


def _r(ap):
    """View an fp32 AP as float32r for full-rate PE matmuls."""
    return ap.bitcast(F32R)


def _bcast(ap1d, p=P):
    """Partition-broadcast AP (stride-0 partition dim) for DMA replication."""
    return bass.AP(
        tensor=ap1d.tensor,
        offset=ap1d.offset,
        ap=[[0, p]] + [list(x) for x in ap1d.ap],
    )


class _Bacc(bacc.Bacc):
    """Bacc whose ACT-table chooser pins Exp and Ln to the one table set
    that contains both (natural_log_exp_and_others), so the kernel needs a
    single ACT_TABLE_LOAD instead of thrashing ~2.7us reloads between the
    softmax Exps and the LayerNorm rsqrt Ln/Exp pair."""

    def insert_act_table_loads(self):
        from concourse.hw_specs import get_activation_tables
        from concourse.bacc import _bass_rust as _br

        has_activation = any(
            isinstance(i, mybir.InstActivation)
            for b in self.main_func.blocks
            for i in b.instructions
        )
        if not has_activation:
            return
        strip = {AF.Exp, AF.Ln}
        tables = [
            (name, funcs if name == "natural_log_exp_and_others"
             else funcs - strip)
            for name, funcs in get_activation_tables(self.m.arch).items()
        ]
        _br.insert_act_table_loads(self, tables)


def build_program(repeat=1):
    nc = _Bacc("TRN2", target_bir_lowering=False, debug=False)

    x = [
        nc.dram_tensor(f"x{i}", [D, NF[i]], F32, kind="ExternalInput").ap()
        for i in range(S)
    ]
    w_dram = {}
    for nm in ("qw", "kw", "vw", "ow"):
        w_dram[nm] = nc.dram_tensor(nm, [S, D, D], F32, kind="ExternalInput").ap()
    b_dram = {}
    for nm in ("qb", "kb", "vb", "ob", "ln_g", "ln_b"):
        b_dram[nm] = nc.dram_tensor(nm, [S, D], F32, kind="ExternalInput").ap()
    y = [
        nc.dram_tensor(f"y{i}", [D, NQ[i]], F32, kind="ExternalOutput").ap()
        for i in range(S)
    ]

    with tile.TileContext(nc) as tc:
        if repeat == 1:
            with ExitStack() as ctx:
                _emit(tc, ctx, x, w_dram, b_dram, y)
        else:
            # hardware loop: repeat the whole computation inside one NEFF so
            # on-device time dominates the ~80 ms axon dispatch overhead
            with tc.For_i(0, repeat, 1):
                with ExitStack() as ctx:
                    _emit(tc, ctx, x, w_dram, b_dram, y)
    nc.compile()
    return nc


def _emit(tc, ctx, x, w_dram, b_dram, y):
    nc = tc.nc

    singles = ctx.enter_context(tc.tile_pool(name="singles", bufs=1))
    tpsum = ctx.enter_context(tc.tile_pool(name="tpsum", bufs=2, space="PSUM"))
    spsum = ctx.enter_context(tc.tile_pool(name="spsum", bufs=2, space="PSUM"))
    apsum = ctx.enter_context(tc.tile_pool(name="apsum", bufs=4, space="PSUM"))

    ident = singles.tile([P, P], F32, tag="ident")
    make_identity(nc, ident)

    eps_t = singles.tile([P, 1], F32, tag="eps")
    nc.vector.memset(eps_t, EPS)

    # ---- biases ----
    bk = {}
    for nm in ("qb", "kb"):
        for s in range(S):
            t = singles.tile([P, 2], F32, tag=f"bk_{nm}{s}")
            nc.gpsimd.dma_start(
                out=t, in_=b_dram[nm][s].rearrange("(po pi) -> pi po", pi=P)
            )
            bk[(nm, s)] = t
    bb = {}
    for nm in ("vb", "ob", "ln_g", "ln_b"):
        for s in range(S):
            t = singles.tile([P, D], F32, tag=f"bb_{nm}{s}")
            nc.gpsimd.dma_start(out=t, in_=_bcast(b_dram[nm][s]))
            bb[(nm, s)] = t

    # ---- transposed weights:  wT[(kind,s)] is [Din_pi, Din_po, Dout] ----
    # scale-0 weights first: the first projections depend only on them and
    # the first X0 chunks, so the PE starts real work ~8us earlier
    KINDS = [(k, s) for s in range(S) for k in ("kw", "vw", "qw", "ow")]
    with tc.tile_pool(name="wstage", bufs=6) as wstage, \
            tc.tile_pool(name="xldstage", bufs=2) as xldstage, \
            tc.tile_pool(name="xstage", bufs=1) as xstage:
        # queue all weight DMAs up front so they pipeline through the DMA
        # engines while the PE chews transposes
        wsb = {}
        for kind, s in KINDS:
            t = wstage.tile([P, 2, D], F32, tag="wload", name="wload")
            nc.sync.dma_start(
                out=t, in_=w_dram[kind][s].rearrange("(po pi) f -> pi po f", pi=P)
            )
            wsb[(kind, s)] = t
        wT = {}

        def emit_transposes(scale):
            for kind, s in KINDS:
                if s != scale:
                    continue
                wt = singles.tile([P, 2, D], F32R, tag=f"wT_{kind}{s}",
                                  name="wt")
                for a in range(2):       # Din outer
                    for bo in range(2):  # Dout outer
                        pt = tpsum.tile([P, 512], F32, tag="t", name="t")[:, :P]
                        nc.tensor.transpose(
                            pt, wsb[(kind, s)][:, bo, a * P:(a + 1) * P], ident
                        )
                        if kind == "ow":
                            # fold the "combined / S" into the output proj
                            nc.vector.tensor_scalar_mul(
                                out=wt[:, a, bo * P:(bo + 1) * P], in0=pt,
                                scalar1=1.0 / S,
                            )
                        else:
                            nc.vector.tensor_copy(
                                out=wt[:, a, bo * P:(bo + 1) * P], in_=pt
                            )
                wT[(kind, s)] = wt

        # ---- projections ----
        # queries of all scales are packed into one global column space so
        # every attention chunk is >=256 wide (full-rate f32r matmuls)
        QOFF = [0, NQ[0], NQ[0] + NQ[1]]           # [0, 2048, 2560]
        NQT = NQ[0] + NQ[1] + NQ[2]                # 2688
        qTcat = singles.tile([P, 2, NQT], F32R, tag="qTcat")

        kT, vh = {}, {}
        emit_transposes(0)
        emit_transposes(1)
        emit_transposes(2)
        for s in range(S):
            # DMA X in chunks, rounding each into a persistent f32r tile
            # (f32r matmul operands need a compute-engine producer)
            xs = xstage.tile([P, 2, NF[s]], F32R, tag="x")
            for c0 in range(0, NF[s], CK):
                w = min(CK, NF[s] - c0)
                xld = xldstage.tile([P, 2, CK], F32, tag="xld", name="xld")[:, :, :w]
                nc.sync.dma_start(
                    out=xld,
                    in_=x[s].rearrange("(po pi) n -> pi po n", pi=P)[:, :, c0:c0 + w],
                )
                nc.vector.tensor_copy(out=xs[:, :, c0:c0 + w], in_=xld)

            # K^T_s : [Dout_pi, Dout_po, N]
            kt = singles.tile([P, 2, NF[s]], F32R, tag=f"kT{s}")
            for dt_ in range(2):
                for c0 in range(0, NF[s], CK):
                    w = min(CK, NF[s] - c0)
                    ps = tpsum.tile([P, 512], F32, tag="t", name="t")[:, :w]
                    for a in range(2):
                        nc.tensor.matmul(
                            ps,
                            lhsT=wT[("kw", s)][:, a, dt_ * P:(dt_ + 1) * P],
                            rhs=_r(xs[:, a, c0:c0 + w]),
                            start=(a == 0),
                            stop=(a == 1),
                        )
                    nc.vector.tensor_scalar_add(
                        out=kt[:, dt_, c0:c0 + w],
                        in0=ps,
                        scalar1=bk[("kb", s)][:, dt_:dt_ + 1],
                    )
            kT[s] = kt

            # V_s padded to 258 cols: col 256 = ones (softmax denominator
            # accumulator), col 257 = zeros (fp32r matmuls need an even
            # innermost dst count; f32r Memset is not a valid ISA inst)
            vt = singles.tile([P, NF[s] // P, D + 2], F32R, tag=f"v{s}")
            for rt in range(NF[s] // P):
                ps = tpsum.tile([P, 512], F32, tag="t", name="t")[:, :D]
                for a in range(2):
                    nc.tensor.matmul(
                        ps,
                        lhsT=_r(xs[:, a, rt * P:(rt + 1) * P]),
                        rhs=wT[("vw", s)][:, a, :],
                        start=(a == 0),
                        stop=(a == 1),
                    )
                nc.vector.tensor_add(
                    out=vt[:, rt, :D], in0=ps, in1=bb[("vb", s)]
                )
                nc.vector.tensor_scalar(
                    out=vt[:, rt, D:D + 1], in0=eps_t,
                    scalar1=0.0, scalar2=1.0, op0=ALU.mult, op1=ALU.add,
                )
                nc.vector.tensor_scalar(
                    out=vt[:, rt, D + 1:D + 2], in0=eps_t,
                    scalar1=0.0, scalar2=None, op0=ALU.mult,
                )
            vh[s] = vt

            # Q^T_s into the packed query tile
            for dt_ in range(2):
                for c0 in range(0, NQ[s], CK):
                    w = min(CK, NQ[s] - c0)
                    ps = tpsum.tile([P, 512], F32, tag="t", name="t")[:, :w]
                    for a in range(2):
                        nc.tensor.matmul(
                            ps,
                            lhsT=wT[("qw", s)][:, a, dt_ * P:(dt_ + 1) * P],
                            rhs=_r(xs[:, a, c0:c0 + w]),
                            start=(a == 0),
                            stop=(a == 1),
                        )
                    nc.vector.tensor_scalar_add(
                        out=qTcat[:, dt_, QOFF[s] + c0:QOFF[s] + c0 + w],
                        in0=ps,
                        scalar1=bk[("qb", s)][:, dt_:dt_ + 1],
                    )

    # ---- attention over the packed query space ----
    etp = ctx.enter_context(tc.tile_pool(name="etp", bufs=3))
    combp = ctx.enter_context(tc.tile_pool(name="combp", bufs=3))
    ctp = ctx.enter_context(tc.tile_pool(name="ctp", bufs=6))
    op_ = ctx.enter_context(tc.tile_pool(name="op", bufs=8))
    xrp = ctx.enter_context(tc.tile_pool(name="xrp", bufs=6))
    ysbp = ctx.enter_context(tc.tile_pool(name="ysbp", bufs=4))
    miscp = ctx.enter_context(tc.tile_pool(name="miscp", bufs=8))

    QOFF = [0, NQ[0], NQ[0] + NQ[1]]
    CHUNKS = [(k * 384, 384) for k in range(7)]

    def sub_scale(g0):
        """global col -> (scale, local col)"""
        for s in range(S - 1, -1, -1):
            if g0 >= QOFF[s]:
                return s, g0 - QOFF[s]
        raise AssertionError

    pending = []

    def drain(k):
        for _ in range(min(k, len(pending))):
            pending.pop(0)()

    def drain_keep(n):
        # drain all but n items: keeps PE filler work in reserve for the
        # next chunk-boundary stall (av-psum recycling waits on DVE)
        while len(pending) > n:
            pending.pop(0)()

    def stage_a(comb, qs, g0):
        def emit():
            s, lc0 = sub_scale(g0)
            cT = ctp.tile([P, 2, P], F32R, tag="cT", name="cT")
            for a in range(2):
                pt = tpsum.tile([P, 512], F32, tag="t", name="t")[:, :P]
                nc.tensor.transpose(pt, comb[:, qs, a * P:(a + 1) * P], ident)
                nc.vector.tensor_copy(out=cT[:, a, :], in_=pt)
            xr = xrp.tile([P, 2, P], F32, tag="xr", name="xr")
            nc.sync.dma_start(
                out=xr,
                in_=x[s].rearrange("(po pi) n -> pi po n", pi=P)[:, :, lc0:lc0 + P],
            )
            ops = tpsum.tile([P, 512], F32, tag="t", name="t")[:, :D]
            for a in range(2):
                nc.tensor.matmul(
                    ops,
                    lhsT=cT[:, a, :],
                    rhs=wT[("ow", s)][:, a, :],
                    start=(a == 0),
                    stop=(a == 1),
                )
            o = op_.tile([P, D], F32, tag="o", name="o")
            nc.vector.tensor_add(out=o, in0=ops, in1=bb[("ob", s)])
            for a in range(2):
                xt = tpsum.tile([P, 512], F32, tag="t", name="t")[:, :P]
                nc.tensor.transpose(xt, xr[:, a, :], ident)
                nc.vector.tensor_add(
                    out=o[:, a * P:(a + 1) * P],
                    in0=o[:, a * P:(a + 1) * P],
                    in1=xt,
                )
            # LayerNorm over the free dim; rstd = exp(-0.5*ln(var+eps)) so
            # Ln/Exp share the softmax Exp's ACT table set (no reloads)
            stats = miscp.tile([P, 6], F32, tag="st", name="st")
            nc.vector.bn_stats(out=stats, in_=o)
            mv = miscp.tile([P, 2], F32, tag="mv", name="mv")
            nc.vector.bn_aggr(out=mv, in_=stats)
            nc.scalar.activation(
                out=mv[:, 1:2], in_=mv[:, 1:2], func=AF.Ln,
                bias=eps_t, scale=1.0,
            )
            nc.scalar.activation(
                out=mv[:, 1:2], in_=mv[:, 1:2], func=AF.Exp, scale=-0.5,
            )
            nc.vector.tensor_scalar(
                out=o, in0=o,
                scalar1=mv[:, 0:1], scalar2=mv[:, 1:2],
                op0=ALU.subtract, op1=ALU.mult,
            )
            nc.vector.tensor_mul(out=o, in0=o, in1=bb[("ln_g", s)])
            nc.vector.tensor_add(out=o, in0=o, in1=bb[("ln_b", s)])
            pending.append(stage_b(o, g0))
        return emit

    def stage_b(o, g0):
        def emit():
            s, lc0 = sub_scale(g0)
            ysb = ysbp.tile([P, 2, P], F32, tag="ysb", name="ysb")
            for a in range(2):
                yt = tpsum.tile([P, 512], F32, tag="t", name="t")[:, :P]
                nc.tensor.transpose(yt, o[:, a * P:(a + 1) * P], ident)
                nc.vector.tensor_copy(out=ysb[:, a, :], in_=yt)
            nc.sync.dma_start(
                out=y[s].rearrange("(po pi) n -> pi po n", pi=P)[:, :, lc0:lc0 + P],
                in_=ysb,
            )
        return emit

    for g0, W in CHUNKS:
        nqs = W // P
        comb = combp.tile([P, 4, D], F32, tag="comb", name="comb")[:, :nqs, :]
        for j in range(S):
            ktiles = NF[j] // P
            avp = [
                apsum.tile([P, 512], F32, tag="av", name="av")[:, :D + 2]
                for _ in range(nqs)
            ]
            for kt_ in range(ktiles):
                sps = spsum.tile([P, 512], F32, tag="s", name="s")[:, :W]
                for a in range(2):
                    nc.tensor.matmul(
                        sps,
                        lhsT=kT[j][:, a, kt_ * P:(kt_ + 1) * P],
                        rhs=qTcat[:, a, g0:g0 + W],
                        start=(a == 0),
                        stop=(a == 1),
                    )
                et = etp.tile([P, 512], F32R, tag="et", name="et")[:, :W]
                nc.scalar.activation(out=et, in_=sps, func=AF.Exp, scale=SCALE)
                for qs in range(nqs):
                    nc.tensor.matmul(
                        avp[qs],
                        lhsT=et[:, qs * P:(qs + 1) * P],
                        rhs=vh[j][:, kt_, :],
                        start=(kt_ == 0),
                        stop=(kt_ == ktiles - 1),
                    )
            for qs in range(nqs):
                rec = miscp.tile([P, 1], F32, tag="rec", name="rec")
                nc.vector.reciprocal(out=rec, in_=avp[qs][:, D:D + 1])
                if j == 0:
                    nc.vector.tensor_scalar_mul(
                        out=comb[:, qs, :], in0=avp[qs][:, :D], scalar1=rec
                    )
                else:
                    tmp = miscp.tile([P, D], F32, tag="ntmp", name="ntmp")
                    nc.vector.tensor_scalar_mul(
                        out=tmp, in0=avp[qs][:, :D], scalar1=rec
                    )
                    nc.vector.tensor_add(
                        out=comb[:, qs, :], in0=comb[:, qs, :], in1=tmp
                    )
            # overlap deferred out-proj work of the previous chunk with this
            # chunk's attention so the in-order PE never stalls on LN chains
            if j < S - 1:
                drain_keep(2)
        for qs in range(nqs):
            pending.append(stage_a(comb, qs, g0 + qs * P))
        drain_keep(4)
    drain(len(pending) + 64)
    while pending:
        drain(len(pending) + 64)


# ---------------------------------------------------------------------------
# host-side sharding / execution
# ---------------------------------------------------------------------------

_CACHE = {}


def _shard_inputs(inputs):
    feats = [np.ascontiguousarray(np.asarray(inputs[f"feat{i}"], np.float32))
             for i in range(S)]
    consts = {
        nm: np.ascontiguousarray(np.asarray(inputs[nm], np.float32))
        for nm in ("qw", "kw", "vw", "ow", "qb", "kb", "vb", "ob", "ln_g", "ln_b")
    }
    in_maps = []
    for c in range(8):
        b, h = c // 2, c % 2
        m = dict(consts)
        for i in range(S):
            xi = feats[i][b].reshape(D, NF[i])
            if h == 1:
                xi = np.concatenate([xi[:, NF[i] // 2:], xi[:, :NF[i] // 2]], axis=1)
            m[f"x{i}"] = np.ascontiguousarray(xi)
        in_maps.append(m)
    return in_maps


def _unshard_outputs(results):
    outs = []
    for i in range(S):
        full = np.empty((B, D, NF[i]), np.float32)
        for c in range(8):
            b, h = c // 2, c % 2
            full[b][:, h * NQ[i]:(h + 1) * NQ[i]] = results[c][f"y{i}"]
        outs.append(full.reshape(B, D, HW[i], HW[i]))
    return tuple(outs)


def get_nc():
    if "nc" not in _CACHE:
        _CACHE["nc"] = build_program()
    return _CACHE["nc"]


def _get_runner():
    if "runner" not in _CACHE:
        _CACHE["runner"] = _build_runner(get_nc())
    return _CACHE["runner"]


def _build_runner(nc):
    """Build a jitted 8-core SPMD executor for a Bass program.

    Mirrors bass2jax.run_bass_via_pjrt's multi-core path, but caches the
    jitted callable so repeat calls don't re-trace/re-compile. No donation:
    the kernel writes every output element, so fresh result buffers are fine.
    """
    import jax
    from jax.sharding import Mesh, PartitionSpec
    from jax.experimental.shard_map import shard_map
    from concourse import bass2jax, mybir

    bass2jax.install_neuronx_cc_hook()

    partition_name = (
        nc.partition_id_tensor.name if nc.partition_id_tensor else None
    )
    in_names, out_names, out_avals, zero_outs = [], [], [], []
    for alloc in nc.m.functions[0].allocations:
        if not isinstance(alloc, mybir.MemoryLocationSet):
            continue
        name = alloc.memorylocations[0].name
        if alloc.kind == "ExternalInput":
            if name != partition_name:
                in_names.append(name)
        elif alloc.kind == "ExternalOutput":
            shape = tuple(alloc.tensor_shape)
            dtype = mybir.dt.np(alloc.dtype)
            out_names.append(name)
            out_avals.append(jax.core.ShapedArray(shape, dtype))
            zero_outs.append(np.zeros(shape, dtype))
    n_params = len(in_names)
    all_names = in_names + out_names
    if partition_name is not None:
        all_names = all_names + [partition_name]

    def _body(*args):
        operands = list(args)
        if partition_name is not None:
            operands.append(bass2jax.partition_id_tensor())
        outs = bass2jax._bass_exec_p.bind(
            *operands,
            out_avals=tuple(out_avals),
            in_names=tuple(all_names),
            out_names=tuple(out_names),
            lowering_input_output_aliases=(),
            sim_require_finite=True,
            sim_require_nnan=True,
            nc=nc,
        )
        return tuple(outs)

    n_cores = 8
    devices = jax.devices()[:n_cores]
    mesh = Mesh(np.asarray(devices), ("core",))
    specs = (PartitionSpec("core"),) * (n_params + len(out_names))
    sharded = jax.jit(
        shard_map(
            _body, mesh=mesh,
            in_specs=specs,
            out_specs=(PartitionSpec("core"),) * len(out_names),
            check_rep=False,
        ),
        keep_unused=True,
    )

    concat_zeros = [
        np.zeros((n_cores * z.shape[0], *z.shape[1:]), z.dtype) for z in zero_outs
    ]
    zeros_dev = [jax.device_put(z, jax.NamedSharding(mesh, PartitionSpec("core")))
                 for z in concat_zeros]

    def prepare(in_maps):
        assert len(in_maps) == n_cores
        concat_in = [
            np.concatenate([np.asarray(in_maps[c][nm]) for c in range(n_cores)],
                           axis=0)
            for nm in in_names
        ]
        return [
            jax.device_put(a, jax.NamedSharding(mesh, PartitionSpec("core")))
            for a in concat_in
        ]

    def run(dev_in):
        out_arrs = sharded(*dev_in, *zeros_dev)
        jax.block_until_ready(out_arrs)
        return out_arrs

    def unpack(out_arrs):
        return [
            {
                nm: np.asarray(out_arrs[i]).reshape(n_cores, *out_avals[i].shape)[c]
                for i, nm in enumerate(out_names)
            }
            for c in range(n_cores)
        ]

    return (prepare, run, unpack)


def kernel(**inputs):
    prepare, run, unpack = _get_runner()
    in_maps = _shard_inputs(inputs)
    results = unpack(run(prepare(in_maps)))
    return _unshard_outputs(results)


if __name__ == "__main__":
    nc = build_program()
    print("program built ok")


def measure_hw_ns(inputs, r_small=256, r_big=1024, samples=12):
    """Measure on-device kernel time by differencing two hardware-loop
    builds (r_big vs r_small iterations inside one NEFF). The ~80 ms axon
    dispatch overhead and any per-invocation constants cancel; samples are
    interleaved so tunnel-latency drift affects both builds equally."""
    import time as _time

    in_maps = _shard_inputs(inputs)

    def get_runner(r):
        key = ("rep", r)
        if key not in _CACHE:
            _CACHE[key] = _build_runner(build_program(repeat=r))
        return _CACHE[key]

    ra, rb = get_runner(r_small), get_runner(r_big)
    dev_a = ra[0](in_maps)
    dev_b = rb[0](in_maps)
    ra[1](dev_a)  # warmup / compile
    rb[1](dev_b)
    ta, tb = [], []
    for _ in range(samples):
        t0 = _time.perf_counter()
        ra[1](dev_a)
        ta.append(_time.perf_counter() - t0)
        t0 = _time.perf_counter()
        rb[1](dev_b)
        tb.append(_time.perf_counter() - t0)
    best_a, best_b = min(ta), min(tb)
    per_iter = (best_b - best_a) / (r_big - r_small)
    outs = rb[2](rb[1](dev_b))
    return per_iter * 1e9, best_a, best_b, outs
